# revision 12
# baseline (speedup 1.0000x reference)
"""Multi-head transposed (channel) attention kernel for Trainium2.

Reference computation (per batch b, head h, c=32 channels, n=65536 spatial):
    q,k,v = split(qkv)                       # each [32, n] per (b,h)
    qh = q / max(||q||_row, 1e-12)           # L2 normalize over n
    kh = k / max(||k||_row, 1e-12)
    S = (qh @ kh.T) * temperature[h]         # [32, 32]
    A = softmax(S, axis=-1)
    out = A @ v                              # [32, n]

Sharding: 24 (b,h) pairs over 8 cores = 3 pairs/core, stacked on 96
partitions.  q,k are cast to fp8 e4m3 on the host (error largely cancels
in the normalized Gram matmuls) and pre-transposed into the SBUF tile
layout [chunk, 128 (spatial), sub, 192 (q|k)] so pass-1 loads are plain
contiguous DMAs.  v is ALSO cast to fp8, paired with a per-head
error-feedback row: corr[j] = 16 * mean over the head's 32 channels of
(v - fp8(v)).  Since softmax rows sum to 1, out = A@fp8(v) + A@r where
A@r ~= colmean(r); the kernel adds that term exactly through one extra
contraction row per head in the pass-2 matmul (lhsT row = rowsum/16,
rhs row = corr), so the fp8 quantization of v cancels to ~2e-4.

Per core (one HWDGE ring, strict FIFO = natural priority):
  pass 1: 8x 1.57MB qk chunk loads; per 256-spatial slice two fp8
          DoubleRow matmuls accumulate [Gq | S | Gk] into one PSUM bank
          (contraction over spatial on partitions, 2 k-tiles per MM).
          v loads queue right behind qk on the same ring and land in
          SBUF (resident) while the PE finishes the Gram.
  logits: rq=temp/sqrt(diag Gq), rk=1/sqrt(diag Gk); scale S rows by rq,
          32-block DVE stream-transpose (attention is per-head block
          diagonal, so blockwise transpose == real transpose after the
          block mask), exp fuses the rk scale, block mask zeroes the
          cross terms.  Rowsums via a ones-RHS matmul land directly on
          partitions; a second tiny matmul vs a head-indicator matrix
          builds the correction lhsT rows.  PE keep-warm filler bridges
          the chain so HAM stays 8/8.
  pass 2: out = attn^T-ext @ [v8; corr] in N=2048 PSUM tiles (4 banks,
          2 bufs); PSUM->SBUF copies (with 1/rowsum scale) alternate
          DVE/ACT; 16 stores stream from the otherwise-idle sync ring.
"""

import ml_dtypes
import numpy as np

import concourse.bass as bass
import concourse.tile as tile
from concourse import bacc, mybir
from concourse.bass_utils import run_bass_kernel_spmd

F32 = mybir.dt.float32
F16 = mybir.dt.float16
F8 = mybir.dt.float8e4

B = 4
HD = 6
CH = 32          # channels per head
HW = 65536       # spatial size (256*256)
P = 96           # partition stack: 3 pairs * 32 channels
P2 = 192         # q-stack + k-stack channels
N_CORES = 8
PAIRS_PER_CORE = 3

USE_V8 = True    # v in fp8 + per-head error-feedback row
USE_DR = True    # fp8 DoubleRow matmuls in pass 1

PC = P + PAIRS_PER_CORE if USE_V8 else P   # contraction rows in pass 2

SUB = 128
NCH1 = 16                  # pass-1 qk chunks
SPC = HW // NCH1 // SUB    # 64 subs per chunk
VCH = HW // 8              # v load chunk: 8 loads of [PC, 8192]
NF = 512                   # one PSUM bank of fp32
TILE2 = 2048               # pass-2 PSUM tile (4 banks)
OCH = 4096                 # out store chunk


def build_nc():
    nc = bacc.Bacc("TRN2", target_bir_lowering=False, debug=False,
                   num_devices=N_CORES)
    qk_d = nc.dram_tensor("qk", [NCH1, SUB, SPC, P2], F8,
                          kind="ExternalInput").ap()
    v_dt = F8 if USE_V8 else F16
    v_d = nc.dram_tensor("vc", [PC, HW], v_dt, kind="ExternalInput").ap()
    dmask_d = nc.dram_tensor("dmask", [P, 3, P], F32,
                             kind="ExternalInput").ap()
    bmask_d = nc.dram_tensor("bmask", [P, P], F16, kind="ExternalInput").ap()
    hm_d = nc.dram_tensor("hm", [P, PAIRS_PER_CORE], F16,
                          kind="ExternalInput").ap()
    ones_d = nc.dram_tensor("ones1", [P, 1], F16, kind="ExternalInput").ap()
    t_d = nc.dram_tensor("tvec", [P, 1], F32, kind="ExternalInput").ap()
    o_d = nc.dram_tensor("out", [P, HW], F16, kind="ExternalOutput").ap()

    with tile.TileContext(nc) as tc:
        _body(nc, tc, qk_d, v_d, dmask_d, bmask_d, hm_d, ones_d, t_d, o_d)
    nc.compile()
    return nc


def _body(nc, tc, qk_d, v_d, dmask_d, bmask_d, hm_d, ones_d, t_d, o_d):
    Exp = mybir.ActivationFunctionType.Exp
    Copy = mybir.ActivationFunctionType.Copy
    add = mybir.AluOpType.add
    mult = mybir.AluOpType.mult
    DR = mybir.MatmulPerfMode.DoubleRow if USE_DR else None

    with (
        tc.tile_pool(name="const", bufs=1) as constp,
        tc.tile_pool(name="persist", bufs=1) as pp,
    ):
        dmask = constp.tile([P, 3, P], F32)
        bmask = constp.tile([P, P], F16)
        hm = constp.tile([P, PAIRS_PER_CORE], F16)
        ones1 = constp.tile([P, 1], F16)
        tv = constp.tile([P, 1], F32)
        # consts ride the scalar ring; qk/v/out own the sync ring
        nc.scalar.dma_start(out=dmask[:, :, :], in_=dmask_d[:, :, :])
        nc.scalar.dma_start(out=bmask[:, :], in_=bmask_d[:, :])
        nc.scalar.dma_start(out=hm[:, :], in_=hm_d[:, :])
        nc.scalar.dma_start(out=ones1[:, :], in_=ones_d[:, :])
        nc.scalar.dma_start(out=tv[:, :], in_=t_d[:, :])

        # warm both ACT tables used by the logits chain
        warm = pp.tile([1, 2], F32)
        nc.gpsimd.memset(warm[:, :], 1.0)
        nc.scalar.sqrt(out=warm[:, 0:1], in_=warm[:, 0:1])
        nc.scalar.activation(out=warm[:, 1:2], in_=warm[:, 1:2], func=Exp)
        # zero operand for the PE keep-warm filler
        zt = pp.tile([P, NF], F16)
        nc.gpsimd.memset(zt[:, :], 0.0)
        # attn^T weights padded to 128 columns so LDWEIGHTS gets FWL
        # (fast weight load needs a full-128-col weight); cols 96:128
        # feed unread PSUM partitions and stay zero
        E_full = pp.tile([PC, SUB], F16)
        nc.gpsimd.memset(E_full[:, :], 0.0)
        # zero fp8 rhs for the pass-2 PE-duty pad matmuls
        zv8 = pp.tile([PC, NF], F8)
        nc.gpsimd.memset(zv8[:, :], 0.0)

        # one PSUM bank accumulates [Gq | S | Gk], each [96, 96]
        psS_cm = tc.tile_pool(name="psS", bufs=1, space="PSUM")
        psS_p = psS_cm.__enter__()
        acc = psS_p.tile([P, 3, P], F32)

        # v tiles: pool opened BEFORE io1 so the stack allocator gives
        # them addresses disjoint from the qk streaming tiles — otherwise
        # the v loads inherit a WAR dependency on every pass-1 matmul
        iov_cm = tc.tile_pool(name="iov", bufs=1)
        iov = iov_cm.__enter__()
        v_dt = F8 if USE_V8 else F16
        vtiles = [iov.tile([PC, VCH], v_dt, tag=f"v{i}", bufs=1, name=f"vt{i}")
                  for i in range(8)]

        # ---------------- pass 1: Gq, S, Gk ----------------
        with tc.tile_pool(name="io1", bufs=8) as io1:
            for t in range(NCH1):
                qkT = io1.tile([SUB, SPC, P2], F8, tag="qkT")
                nc.sync.dma_start(out=qkT[:, :, :], in_=qk_d[t])
                if USE_DR:
                    for s in range(SPC // 2):
                        first = (t == 0 and s == 0)
                        last = (t == NCH1 - 1 and s == SPC // 2 - 1)
                        sl = slice(2 * s, 2 * s + 2)
                        nc.tensor.matmul(
                            acc[:, 0:2, :],
                            lhsT=qkT[:, sl, 0:P], rhs=qkT[:, sl, :],
                            start=first, stop=last, perf_mode=DR,
                            skip_group_check=True)
                        nc.tensor.matmul(
                            acc[:, 2, :],
                            lhsT=qkT[:, sl, P:P2], rhs=qkT[:, sl, P:P2],
                            start=first, stop=last, perf_mode=DR,
                            skip_group_check=True)
                else:
                    for s in range(SPC):
                        first = (t == 0 and s == 0)
                        last = (t == NCH1 - 1 and s == SPC - 1)
                        nc.tensor.matmul(
                            acc[:, 0:2, :],
                            lhsT=qkT[:, s, 0:P], rhs=qkT[:, s, :],
                            start=first, stop=last, skip_group_check=True)
                        nc.tensor.matmul(
                            acc[:, 2, :],
                            lhsT=qkT[:, s, P:P2], rhs=qkT[:, s, P:P2],
                            start=first, stop=last, skip_group_check=True)

        # v loads queue on the sync ring right behind the qk chunks and
        # land while the PE drains the Gram accumulation
        for i in range(8):
            nc.sync.dma_start(out=vtiles[i][:, :],
                              in_=v_d[:, i * VCH:(i + 1) * VCH])

        # ---------------- norms + logits + softmax ----------------
        # (chain ops emitted BEFORE the filler matmuls so their waits
        # bind to the last Gram matmul, not the fillers)
        rinv = pp.tile([P, 1], F32)
        with tc.tile_pool(name="psC", bufs=1, space="PSUM") as psC:
            dm = pp.tile([P, 3, P], F32)
            nr = pp.tile([P, 3], F32)
            rr = pp.tile([P, 3], F32)
            rq2 = pp.tile([P, 1], F32)
            As = pp.tile([P, P], F32)
            AsT = pp.tile([P, P], F32)
            Et = pp.tile([P, P], F16)

            zn = pp.tile([P, 3], F16)

            # diag(Gq), diag(Gq), diag(Gk) -> [96, 3] in one sweep
            nc.vector.tensor_mul(out=dm[:, :, :], in0=acc[:, :, :],
                                 in1=dmask[:, :, :])
            # tiny fp16 cast whose only job is to anchor the PE keep-warm
            # filler AFTER pass 1 (the scheduler hoists dep-free matmuls)
            nc.vector.tensor_copy(out=zn[:, :], in_=dm[:, :, 0])
            nc.vector.tensor_reduce(out=nr[:, :], in_=dm[:, :, :],
                                    axis=mybir.AxisListType.X, op=add)
            nc.scalar.sqrt(out=nr[:, :], in_=nr[:, :])
            nc.vector.reciprocal(out=rr[:, :], in_=nr[:, :])
            # rq2 = temp / |q_c|
            nc.vector.tensor_mul(out=rq2[:, :], in0=rr[:, 0:1],
                                 in1=tv[:, :])
            # row scale in [c,d] layout (on DVE, so the ACT engine's Exp
            # table load hides here), then 32-blockwise transpose
            # (attention is block-diagonal so this IS the transpose)
            nc.vector.tensor_scalar(out=As[:, :], in0=acc[:, 1, :],
                                    scalar1=rq2[:, :], scalar2=None,
                                    op0=mult)
            nc.vector.transpose(out=AsT[:, :], in_=As[:, :])
            # exp fuses the 1/|k_d| partition scale; block mask
            # zeroes the cross-head garbage
            nc.scalar.activation(out=Et[:, :], in_=AsT[:, :],
                                 func=Exp, scale=rr[:, 2:3])
            nc.vector.tensor_mul(out=E_full[0:P, 0:P], in0=Et[:, :],
                                 in1=bmask[:, :])

            # PE keep-warm filler: bridges the logits chain so the HAM
            # clock gate stays at 8/8 into pass 2 (results unused);
            # reading zn pins it after pass 1 in the schedule
            with tc.tile_pool(name="psW", bufs=1, space="PSUM") as psW:
                wacc = psW.tile([PAIRS_PER_CORE, NF], F32)
                NW = 7
                for w in range(NW):
                    nc.tensor.matmul(
                        wacc[:, :], lhsT=zn[:, :], rhs=zt[:, :],
                        start=(w == 0), stop=(w == NW - 1),
                        skip_group_check=True)

            # softmax denominators land directly on partitions
            rs_ps = psC.tile([P, 1], F32, tag="rs")
            nc.tensor.matmul(rs_ps[:, :], lhsT=E_full[0:P, 0:P],
                             rhs=ones1[:, :], start=True, stop=True)
            nc.vector.reciprocal(out=rinv[:, :], in_=rs_ps[:, :])
            if USE_V8:
                # correction lhsT rows: rowsum_c/16 on the head's row
                ext_ps = psC.tile([PAIRS_PER_CORE, P], F32, tag="ext")
                nc.tensor.matmul(ext_ps[:, :], lhsT=hm[:, :],
                                 rhs=E_full[0:P, 0:P], start=True,
                                 stop=True)
                nc.vector.tensor_copy(out=E_full[P:PC, 0:P],
                                      in_=ext_ps[:, :])

        # release the accumulator bank so pass 2 gets all 8
        psS_cm.__exit__(None, None, None)

        # ---------------- pass 2: out = attn_ext @ [v8; corr] -----
        with (
            tc.tile_pool(name="ioo", bufs=4) as ioo,
            tc.tile_pool(name="psO", bufs=2, space="PSUM") as psOp,
        ):
            for j in range(HW // TILE2):
                if j % 2 == 0:
                    on = ioo.tile([P, OCH], F16, tag="on")
                vt = vtiles[j // (VCH // TILE2)]
                vof = (j % (VCH // TILE2)) * TILE2
                o_ps = psOp.tile([SUB, TILE2], F32, tag="o")
                NPAD = 1
                for m in range(TILE2 // NF):
                    pad = m == 0
                    nc.tensor.matmul(
                        o_ps[:, m * NF:(m + 1) * NF],
                        lhsT=E_full[:, :],
                        rhs=vt[:, vof + m * NF:vof + (m + 1) * NF],
                        start=True, stop=not pad)
                    if pad:
                        # same-weight zero-rhs accumulates: keep PE duty
                        # high for HAM without an LDWEIGHTS thrash
                        for w in range(NPAD):
                            nc.tensor.matmul(
                                o_ps[:, 0:NF], lhsT=E_full[:, :],
                                rhs=zv8[:, :], start=False,
                                stop=(w == NPAD - 1))
                # two 1024-col copy ops per PSUM tile: the WAR release
                # happens per chunk, so the PE refills banks while the
                # rest of the tile still drains (PSUM is only 2 tiles
                # deep at 2048 cols — monolithic copies serialize the
                # pipeline into copy->matmul->copy chains)
                osl0 = (j % 2) * TILE2
                for h in range(2):
                    hs = slice(h * 1024, (h + 1) * 1024)
                    os2 = slice(osl0 + h * 1024, osl0 + (h + 1) * 1024)
                    if (j // 2) % 2 == 0:
                        nc.vector.tensor_scalar(
                            out=on[:, os2], in0=o_ps[0:P, hs],
                            scalar1=rinv[:, :], scalar2=None, op0=mult)
                    else:
                        nc.scalar.activation(out=on[:, os2],
                                             in_=o_ps[0:P, hs],
                                             func=Copy, scale=rinv[:, :])
                if j % 2 == 1:
                    sl = slice((j // 2) * OCH, (j // 2 + 1) * OCH)
                    nc.sync.dma_start(out=o_d[:, sl], in_=on[:, :])
        iov_cm.__exit__(None, None, None)


_NC_CACHE = {}


def _get_nc():
    if "nc" not in _NC_CACHE:
        _NC_CACHE["nc"] = build_nc()
    return _NC_CACHE["nc"]


def _consts():
    dmask = np.zeros((P, 3, P), dtype=np.float32)
    for c in range(P):
        dmask[c, 0, c] = 1.0
        dmask[c, 1, c] = 1.0
        dmask[c, 2, c] = 1.0
    bmask = np.zeros((P, P), dtype=np.float16)
    for j in range(PAIRS_PER_CORE):
        bmask[CH * j:CH * (j + 1), CH * j:CH * (j + 1)] = 1.0
    hm = np.zeros((P, PAIRS_PER_CORE), dtype=np.float16)
    for j in range(PAIRS_PER_CORE):
        hm[CH * j:CH * (j + 1), j] = 1.0 / 16.0
    ones1 = np.ones((P, 1), dtype=np.float16)
    return dmask, bmask, hm, ones1


def _shard_inputs(qkv, temperature):
    qkv = np.asarray(qkv)
    temp = np.asarray(temperature, dtype=np.float32).reshape(-1)
    C = HD * CH
    q = qkv[:, 0 * C:1 * C].reshape(B, HD, CH, HW)
    k = qkv[:, 1 * C:2 * C].reshape(B, HD, CH, HW)
    v = qkv[:, 2 * C:3 * C].reshape(B, HD, CH, HW)
    dmask, bmask, hm, ones1 = _consts()
    in_maps = []
    for core in range(N_CORES):
        pairs = [divmod(p, HD) for p in
                 range(core * PAIRS_PER_CORE, (core + 1) * PAIRS_PER_CORE)]
        qs = np.concatenate([q[b_, h_] for b_, h_ in pairs], axis=0)
        ks = np.concatenate([k[b_, h_] for b_, h_ in pairs], axis=0)
        qks = np.concatenate([qs, ks], axis=0).astype(ml_dtypes.float8_e4m3)
        # pre-transpose to the SBUF tile layout [chunk, p, sub, ch]
        qks = np.ascontiguousarray(
            qks.reshape(P2, NCH1, SPC, SUB).transpose(1, 3, 2, 0))
        vs = np.concatenate([v[b_, h_] for b_, h_ in pairs], axis=0)
        if USE_V8:
            v8 = vs.astype(ml_dtypes.float8_e4m3)
            r = vs.astype(np.float32) - v8.astype(np.float32)
            corr = 16.0 * r.reshape(PAIRS_PER_CORE, CH, HW).mean(axis=1)
            vcs = np.concatenate(
                [v8, corr.astype(ml_dtypes.float8_e4m3)], axis=0)
        else:
            vcs = vs.astype(np.float16)
        tvec = np.repeat(np.array([temp[h_] for b_, h_ in pairs],
                                  dtype=np.float32), CH).reshape(P, 1)
        in_maps.append({"qk": qks, "vc": vcs, "dmask": dmask,
                        "bmask": bmask, "hm": hm, "ones1": ones1,
                        "tvec": tvec})
    return in_maps


def _gather_output(results):
    out = np.empty((B, HD, CH, HW), dtype=np.float32)
    for core in range(N_CORES):
        o = results[core]["out"]
        for j in range(PAIRS_PER_CORE):
            b_, h_ = divmod(core * PAIRS_PER_CORE + j, HD)
            out[b_, h_] = o[CH * j:CH * (j + 1)].astype(np.float32)
    return out.reshape(B, HD * CH, 256, 256)


def kernel(qkv, temperature):
    in_maps = _shard_inputs(qkv, temperature)
    nc = _get_nc()
    res = run_bass_kernel_spmd(nc, in_maps, list(range(N_CORES)))
    return _gather_output(res.results)


if __name__ == "__main__":
    rng = np.random.default_rng(0)
    qkv = rng.standard_normal((B, 576, 256, 256), dtype=np.float32)
    temp = np.ones((HD, 1, 1), dtype=np.float32)
    out = kernel(qkv=qkv, temperature=temp)
    print("out", out.shape, out.dtype, float(np.abs(out).max()))
